# revision 22
# baseline (speedup 1.0000x reference)
"""Gaussian-kernel weighted sum (retrieval_knn) on 8 Trainium2 NeuronCores.

    out[b] = sum_g w_g * exp(-||x_b - c_g||^2 / 2)

Strategy (data-parallel over the query batch, centers replicated):
  - Fold the whole exponent into ONE bf16 matmul via augmented features:
        t[b,g] = x_b . c_g  +  (ln|w_g| - ||c_g||^2/2)  +  (-||x_b||^2/2)
    with K = 64 + 4 contraction rows:
        [x_hi; 1; 1; qb_hi; qb_lo] . [c_hi; a_hi; a_lo; 1; 1]
    The additive terms ride as hi/lo bf16 pairs (exact to ~1e-4); the
    x.c product is single bf16 (queries and centers rounded to bf16).
    Measured end-to-end absmax error vs the fp64 reference on the actual
    inputs is 2.8e-3 -- 7x inside the 2e-2 gate -- because the output max
    is dominated by nearest-neighbour terms whose exponent error is
    O(2^-9 * sqrt(D)).  Dropping the hi/lo cross-product passes halves PE
    time and removes a 1MB input stream.
  - ScalarE computes exp with accum_out (sum along the free/g axis), so the
    16.8M-element exp never round-trips through another engine; ScalarE's
    4 full-width activations per 128-query chunk are the kernel's floor
    (~130us busy).  Signs of w are handled by sorting centers (positives
    first); the one PSUM tile that straddles the +/- boundary is
    accumulated whole and the small wrong-sign span re-reduced by the
    otherwise-idle VectorE into an extra slot (out = S - 2N).
  - Every accumulator lands in a distinct column of one flat SBUF tensor,
    DMA'd out once; the tiny signed combine across slots happens on the
    host, so the ScalarE stream has no downstream dependencies.
  - Input DMA is split across SP/ACT HWDGE + Pool SWDGE queues so the first
    matmul operands land in ~3us; a short warm-up matmul chain on a zeroed
    tile brings the PE out of its low p-state while the DMAs fly.
"""

import numpy as np
import ml_dtypes

import concourse.bass as bass
import concourse.mybir as mybir
import concourse.tile as tile
from concourse import bacc
from concourse import bass_utils

BF16 = mybir.dt.bfloat16
F32 = mybir.dt.float32
NPBF16 = ml_dtypes.bfloat16

N_CORES = 8
B, G, D = 16384, 8192, 64
BL = B // N_CORES            # queries per core
NB = BL // 128               # 128-row query chunks per core
GT = 512                     # matmul tile width (one PSUM bank)
SUPER = 2048                 # PSUM half (4 banks) = one ACT chunk
NJ = G // SUPER              # g super-chunks
K1 = D + 4                   # contraction dim (x_hi, 1, 1, qb_hi, qb_lo)
N_WARMUP = 22                # PE p-state warm-up matmuls
DVE_TILES = frozenset()      # DVE-summed tiles: any nonempty set stalls the PSUM ping-pong


def _plan(p_boundary):
    """Static plan from the +/- boundary: per-tile ACT signs, plus an
    optional VectorE-reduced sub-span of the straddling tile.

    Returns (tile_signs[NJ], extra) where extra is None or
    (j, lo, hi, weight): reduce ps[:, lo:hi] of tile j into the extra acc
    slot entering the host-side combine with `weight`.
    """
    signs = []
    extra = None
    for j in range(NJ):
        lo, hi = j * SUPER, (j + 1) * SUPER
        if hi <= p_boundary:
            signs.append(1.0)
        elif lo >= p_boundary:
            signs.append(-1.0)
        else:
            d0 = p_boundary - lo          # positive prefix width
            d1 = hi - p_boundary          # negative suffix width
            if d1 <= d0:
                # count tile as positive, subtract the negative suffix twice
                signs.append(1.0)
                extra = (j, d0, SUPER, -2.0)
            else:
                signs.append(-1.0)
                extra = (j, 0, d0, 2.0)
    return signs, extra


def _nslot(plan):
    return NJ + (1 if plan[1] is not None else 0)


def _build(plan):
    signs, extra = plan
    nslot = _nslot(plan)

    nc = bacc.Bacc(
        "TRN2",
        target_bir_lowering=False,
        debug=False,
        enable_asserts=False,
        num_devices=N_CORES,
    )

    d_r1 = nc.dram_tensor("r1", [K1, G], BF16, kind="ExternalInput")
    d_l1 = nc.dram_tensor("l1", [K1, BL], BF16, kind="ExternalInput")
    d_out = nc.dram_tensor("out", [128, NB * nslot], F32, kind="ExternalOutput")

    with tile.TileContext(nc) as tc:
        from contextlib import ExitStack

        with ExitStack() as ctx:
            cpool = ctx.enter_context(tc.tile_pool(name="const", bufs=1))
            psum_pool = ctx.enter_context(
                tc.tile_pool(name="psum", bufs=2, space="PSUM")
            )

            r1 = cpool.tile([K1, G], BF16)
            l1 = cpool.tile([K1, BL], BF16)
            acc = cpool.tile([128, NB * nslot], F32)
            wt = cpool.tile([128, 64], BF16)

            # Input DMA split across three queues (SP + ACT HWDGE, Pool
            # SWDGE), ordered so super-chunk j's operands land just before
            # the PE needs them and the ACT queue drains early.
            def gsl(j):
                return slice(j * SUPER, (j + 1) * SUPER)

            nc.scalar.dma_start(r1[:, gsl(0)], d_r1.ap()[:, gsl(0)])
            nc.sync.dma_start(l1[:], d_l1.ap())
            nc.gpsimd.dma_start(r1[:, gsl(1)], d_r1.ap()[:, gsl(1)])
            nc.sync.dma_start(r1[:, gsl(2)], d_r1.ap()[:, gsl(2)])
            nc.sync.dma_start(r1[:, gsl(3)], d_r1.ap()[:, gsl(3)])

            # PE p-state warm-up: tiny matmuls on a zeroed tile while the
            # input DMAs are in flight.  Results land in a PSUM tile that is
            # recycled before the first real accumulation.
            nc.vector.memset(wt[:], 0)
            wps = psum_pool.tile([128, SUPER], F32, tag="ps")
            for _ in range(N_WARMUP):
                nc.tensor.matmul(
                    wps[0:1, 0:64], wt[0:64, 0:1], wt[0:64, 0:64],
                    start=True, stop=True,
                )

            for i in range(NB):
                bs = slice(i * 128, (i + 1) * 128)
                for j in range(NJ):
                    ps = psum_pool.tile([128, SUPER], F32)
                    for k in range(SUPER // GT):
                        goff = j * SUPER + k * GT
                        nc.tensor.matmul(
                            ps[:, k * GT : (k + 1) * GT],
                            l1[:, bs],
                            r1[:, goff : goff + GT],
                            start=True,
                            stop=True,
                        )
                    # exp written in place over the PSUM inputs; only the
                    # per-tile sum (along g) is consumed.  A couple of tiles
                    # per chunk are summed by the otherwise-idle VectorE
                    # instead of the ScalarE accumulator, saving the 187ns
                    # accumulator read on those -- but no more than the
                    # PSUM ping-pong slack tolerates (VectorE holds the
                    # tile ~2.4us past the exp).
                    s = i * nslot
                    dve_sum = j in DVE_TILES and i != NB - 1
                    nc.scalar.activation(
                        ps[:],
                        ps[:],
                        mybir.ActivationFunctionType.Exp,
                        bias=0.0,
                        scale=1.0,
                        accum_out=(
                            None if dve_sum else acc[:, s + j : s + j + 1]
                        ),
                    )
                    if dve_sum:
                        nc.vector.tensor_reduce(
                            acc[:, s + j : s + j + 1],
                            ps[:],
                            mybir.AxisListType.X,
                            mybir.AluOpType.add,
                        )
                    if extra is not None and extra[0] == j:
                        _, lo, hi, _ = extra
                        nc.vector.tensor_reduce(
                            acc[:, s + NJ : s + NJ + 1],
                            ps[:, lo:hi],
                            mybir.AxisListType.X,
                            mybir.AluOpType.add,
                        )
            nc.scalar.dma_start(d_out.ap(), acc[:])

    nc.compile()
    return nc


def _prep(input, inputs, weights):
    """Host-side preprocessing -> (shared in_map pieces, per-core pieces)."""
    x = np.asarray(input, dtype=np.float32)
    c = np.asarray(inputs, dtype=np.float32)
    w = np.asarray(weights, dtype=np.float32)

    # Sort centers: positive weights first.
    order = np.argsort(w < 0, kind="stable")
    c = c[order]
    w = w[order]
    p_boundary = int((w >= 0).sum())

    c64 = c.astype(np.float64)
    absw = np.abs(w.astype(np.float64))
    a = np.where(absw > 0, np.log(np.maximum(absw, 1e-300)), -1e4)
    a = (a - (c64 * c64).sum(1) / 2.0).astype(np.float32)
    a = np.maximum(a, np.float32(-1e4))

    def split(v):
        hi = v.astype(NPBF16).astype(np.float32)
        lo = (v - hi).astype(NPBF16)
        return hi.astype(NPBF16), lo

    a_hi, a_lo = split(a)

    r1 = np.empty((K1, G), dtype=NPBF16)
    r1[0:64] = c.astype(NPBF16).T
    r1[64] = a_hi
    r1[65] = a_lo
    r1[66:68] = np.ones((2, G), dtype=NPBF16)

    plan = _plan(p_boundary)

    per_core = []
    for core in range(N_CORES):
        xs = x[core * BL : (core + 1) * BL]
        qb = -(xs.astype(np.float64) ** 2).sum(1) / 2.0
        qb_hi, qb_lo = split(qb.astype(np.float32))
        l1 = np.empty((K1, BL), dtype=NPBF16)
        l1[0:64] = xs.astype(NPBF16).T
        l1[64:66] = np.ones((2, BL), dtype=NPBF16)
        l1[66] = qb_hi
        l1[67] = qb_lo
        per_core.append({"l1": l1})

    shared = {"r1": r1}
    return shared, per_core, plan


def _gather(results, plan):
    """Host-side signed combine of the per-chunk accumulator slots."""
    signs, extra = plan
    nslot = _nslot(plan)
    sgv = np.array(
        list(signs) + ([extra[3]] if extra is not None else []),
        dtype=np.float64,
    )
    outs = []
    for r in results:
        o = r["out"].astype(np.float64).reshape(128, NB, nslot)
        outs.append((o @ sgv).T.reshape(BL))  # [128, NB] -> queries in order
    return np.concatenate(outs).astype(np.float32)


def kernel(input, inputs, weights):
    shared, per_core, plan = _prep(input, inputs, weights)
    nc = _build(plan)
    in_maps = [{**shared, **pc} for pc in per_core]
    res = bass_utils.run_bass_kernel_spmd(
        nc, in_maps, core_ids=list(range(N_CORES))
    )
    return _gather(res.results, plan)
